# revision 2
# baseline (speedup 1.0000x reference)
"""GATv2 (2 layers) + mean-pool + linear head on 8 Trainium2 NeuronCores.

Sharding: destination nodes are range-partitioned across the 8 cores
(6250 nodes each, padded to 6272 = 49*128). Edges (with self-loops) are
sorted by destination and assigned to the owner of their dst. Per core:

  1. node transforms xl=x@Wl+b, xr=x@Wr-b for the local node slice (PE),
  2. AllGather of xl (source-side transform) so any core can gather any
     source row,
  3. per 128-dst tile: dma_gather of per-edge source rows (split in two
     index banks because gather indices are int16), per-edge scores via
     DVE/ACT, per-dst softmax denominator + weighted message aggregation
     via indicator matmuls on the PE (edges of a tile only reference the
     tile's 128 dsts), normalization folded into the psum read-out.

Softmax is computed without the segment-max shift (scores are O(1); the
shift cancels exactly) and per-dst score terms are dropped (they cancel
in the softmax too). leaky_relu(z) = relu(0.8 z) + 0.2 z with the 0.2*xr
part dropped (per-dst) and 0.2*xl kept.

The mean-pool + final linear run as a tiny per-core [8,64] partial
(indicator matmul with 1/count weights) summed on host.
"""

import sys
import numpy as np

for _p in ("/opt/trn_rl_repo", "/root/.axon_site/_ro/trn_rl_repo"):
    if _p not in sys.path:
        sys.path.insert(0, _p)

import ml_dtypes

BF = ml_dtypes.bfloat16

# Problem constants
N, E, F_IN, H, C, G = 50000, 800000, 128, 4, 64, 8
HC = H * C                      # 256
NCORES = 8
RP = N // NCORES                # 6250 rows per core
RPAD = 6272                     # 49*128
NT = RPAD // 128                # 49 dst/node tiles per core
NPADG = NCORES * RPAD           # 50176 padded global rows
BANKA = 5 * RPAD                # 31360; int16 gather bank split

_CACHE = {}


# ----------------------------------------------------------------- host prep

def _wrap16_rows(a):
    """[T, n] int16 -> [16, T*n//16] in dma_gather index layout per row."""
    T, n = a.shape
    return a.reshape(T, n // 16, 16).transpose(2, 0, 1).reshape(16, T * n // 16)


def _prep_core(sp_all, dl_all, tile_of, CHA, CHB):
    """Build padded per-tile index/dst arrays for one core.

    sp_all: global padded src row per edge (sorted by dst)
    dl_all: dst local row (0..6271) per edge
    tile_of: dl_all // 128
    """
    CH = CHA + CHB
    nA, nB = CHA * 128, CHB * 128
    bank_b = sp_all >= BANKA
    # stable order by (tile, bank)
    key = tile_of * 2 + bank_b
    order = np.argsort(key, kind="stable")
    sp = sp_all[order]
    dl = dl_all[order] - tile_of[order] * 128
    keys = key[order]
    cnt = np.bincount(keys, minlength=2 * NT)
    # position of each edge inside its (tile, bank) group
    starts = np.concatenate([[0], np.cumsum(cnt)[:-1]])
    pos = np.arange(len(sp)) - np.repeat(starts, cnt)
    grp_t = keys >> 1
    grp_b = keys & 1
    assert cnt[0::2].max(initial=0) <= nA, "bank-A overflow; raise CHA"
    assert cnt[1::2].max(initial=0) <= nB, "bank-B overflow; raise CHB"

    idxA = np.zeros((NT, nA), np.int16)
    idxB = np.zeros((NT, nB), np.int16)
    dlp = np.full((NT, CH * 128), -1.0, np.float32)
    a = grp_b == 0
    idxA[grp_t[a], pos[a]] = sp[a].astype(np.int16)
    b = ~a
    idxB[grp_t[b], pos[b]] = (sp[b] - BANKA).astype(np.int16)
    dlp[grp_t[a], pos[a]] = dl[a]
    dlp[grp_t[b], nA + pos[b]] = dl[b]

    idxR = np.where(dlp >= 0, dlp + (np.arange(NT) * 128)[:, None], 0).astype(np.int16)
    # device layouts
    return dict(
        idxA=_wrap16_rows(idxA),
        idxB=_wrap16_rows(idxB),
        idxR=_wrap16_rows(idxR),
        # dl[p, t*CH + j] = dst_local of edge (t, j*128+p)
        dl=dlp.reshape(NT, CH, 128).transpose(2, 0, 1).reshape(128, NT * CH),
    )


def _preprocess(x, edge_index, batch):
    src = np.concatenate([edge_index[0].astype(np.int64),
                          np.arange(N, dtype=np.int64)])
    dst = np.concatenate([edge_index[1].astype(np.int64),
                          np.arange(N, dtype=np.int64)])
    order = np.argsort(dst, kind="stable")
    srcs = src[order]
    dsts = dst[order]
    sp_all = srcs + 22 * (srcs // RP)          # padded global row
    core_lo = np.searchsorted(dsts, np.arange(NCORES + 1) * RP)

    # global uniform chunk counts
    tile_gl = (dsts - (dsts // RP) * RP) // 128 + (dsts // RP) * NT
    bank_b = (sp_all >= BANKA).astype(np.int64)
    cnt = np.bincount(tile_gl * 2 + bank_b, minlength=2 * NCORES * NT)
    CHA = int(-(-cnt[0::2].max() // 128))
    CHB = int(-(-cnt[1::2].max() // 128))

    cores = []
    for c in range(NCORES):
        lo, hi = core_lo[c], core_lo[c + 1]
        dl_all = dsts[lo:hi] - c * RP
        cores.append(_prep_core(sp_all[lo:hi], dl_all, dl_all // 128, CHA, CHB))

    # mean-pool weights: [N, G] one-hot / count, padded + tile-major
    cntg = np.bincount(batch.astype(np.int64), minlength=G).astype(np.float32)
    w = np.zeros((NCORES * RPAD, G), np.float32)
    rows = np.arange(N) + 22 * (np.arange(N) // RP)
    w[rows, batch.astype(np.int64)] = 1.0 / np.maximum(cntg, 1.0)[batch.astype(np.int64)]
    poolw = w.reshape(NCORES, NT, 128, G).transpose(0, 2, 1, 3).reshape(
        NCORES, 128, NT * G)
    return cores, poolw, CHA, CHB


# ---------------------------------------------------------------- bass build

def _build_nc(CHA, CHB):
    from contextlib import ExitStack
    from concourse import bacc, mybir
    from concourse import tile

    F32 = mybir.dt.float32
    BF16 = mybir.dt.bfloat16
    I16 = mybir.dt.int16
    AF = mybir.ActivationFunctionType
    OP = mybir.AluOpType
    CH = CHA + CHB

    nc = bacc.Bacc(None, target_bir_lowering=False, debug=False)
    dp = nc.declare_dram_parameter
    x_sl = dp("x_sl", [RPAD, F_IN], BF16, isOutput=False)
    wl1 = dp("wl1", [F_IN, HC], BF16, isOutput=False)
    wr1 = dp("wr1", [F_IN, HC], BF16, isOutput=False)
    wl2 = dp("wl2", [128, 2, C], BF16, isOutput=False)
    wr2 = dp("wr2", [128, 2, C], BF16, isOutput=False)
    b1rep = dp("b1rep", [128, HC], F32, isOutput=False)
    b2rep = dp("b2rep", [128, C], F32, isOutput=False)
    attrep = dp("attrep", [128, HC], BF16, isOutput=False)
    att2rep = dp("att2rep", [128, C], BF16, isOutput=False)
    idxA_d = dp("idxA", [16, NT * CHA * 8], I16, isOutput=False)
    idxB_d = dp("idxB", [16, NT * CHB * 8], I16, isOutput=False)
    idxR_d = dp("idxR", [16, NT * CH * 8], I16, isOutput=False)
    dl_d = dp("dl", [128, NT * CH], F32, isOutput=False)
    poolw_d = dp("poolw", [128, NT * G], F32, isOutput=False)
    out_pool = dp("out_pool", [G, C], F32, isOutput=True)

    xl1_sl = nc.dram_tensor("xl1_sl", [RPAD, HC], BF16)
    xr1_loc = nc.dram_tensor("xr1_loc", [RPAD, HC], BF16)
    xl1_full = nc.dram_tensor("xl1_full", [NPADG, HC], BF16, addr_space="Shared")
    h1c_sl = [nc.dram_tensor(f"h1c{i}_sl", [RPAD, 128], BF16) for i in range(2)]
    xl2_sl = nc.dram_tensor("xl2_sl", [RPAD, C], F32)
    xr2_loc = nc.dram_tensor("xr2_loc", [RPAD, C], F32)
    xl2_full = nc.dram_tensor("xl2_full", [NPADG, C], F32, addr_space="Shared")

    with tile.TileContext(nc) as tc, ExitStack() as ctx:
        cp = ctx.enter_context(tc.tile_pool(name="consts", bufs=1))
        sb = ctx.enter_context(tc.tile_pool(name="work", bufs=2))

        def cload(name, src_ap, shape, dtype):
            t = cp.tile(shape, dtype, tag=name)
            nc.sync.dma_start(t[:], src_ap)
            return t

        wl1_t = cload("wl1c", wl1[:, :], [F_IN, HC], BF16)
        wr1_t = cload("wr1c", wr1[:, :], [F_IN, HC], BF16)
        wl2_t = cload("wl2c", wl2[:, :, :], [128, 2, C], BF16)
        wr2_t = cload("wr2c", wr2[:, :, :], [128, 2, C], BF16)
        b1_t = cload("b1c", b1rep[:, :], [128, HC], F32)
        b2_t = cload("b2c", b2rep[:, :], [128, C], F32)
        att_t = cload("attc", attrep[:, :], [128, HC], BF16)
        att2_t = cload("att2c", att2rep[:, :], [128, C], BF16)
        dl_t = cload("dlc", dl_d[:, :], [128, NT * CH], F32)
        poolw_t = cload("poolwc", poolw_d[:, :], [128, NT * G], F32)

        iota_i = cp.tile([128, 128], mybir.dt.int32)
        nc.gpsimd.iota(iota_i[:], pattern=[[1, 128]], base=0, channel_multiplier=0)
        iota_f = cp.tile([128, 128], F32)
        nc.vector.tensor_copy(iota_f[:], iota_i[:])

        iA_t = cp.tile([128, NT * CHA * 8], I16)
        iB_t = cp.tile([128, NT * CHB * 8], I16)
        iR_t = cp.tile([128, NT * CH * 8], I16)
        for k in range(8):
            nc.sync.dma_start(iA_t[16 * k:16 * (k + 1), :], idxA_d[:, :])
            nc.sync.dma_start(iB_t[16 * k:16 * (k + 1), :], idxB_d[:, :])
            nc.sync.dma_start(iR_t[16 * k:16 * (k + 1), :], idxR_d[:, :])

        # ---- stage 1: x^T and layer-1 node transforms
        xT = cp.tile([128, RPAD], BF16)
        nc.sync.dma_start_transpose(xT[:], x_sl[:, :])
        psx = ExitStack()
        ctx.callback(psx.close)
        ps = psx.enter_context(tc.tile_pool(name="ps1", bufs=2, space="PSUM"))
        for i in range(NT):
            pa = ps.tile([128, HC], F32, tag="p_nl")
            nc.tensor.matmul(pa[:], xT[:, i * 128:(i + 1) * 128], wl1_t[:],
                             start=True, stop=True)
            ta = sb.tile([128, HC], BF16, tag="t_nl")
            nc.vector.tensor_add(ta[:], pa[:], b1_t[:])
            nc.sync.dma_start(xl1_sl[i * 128:(i + 1) * 128, :], ta[:])
            pb = ps.tile([128, HC], F32, tag="p_nr")
            nc.tensor.matmul(pb[:], xT[:, i * 128:(i + 1) * 128], wr1_t[:],
                             start=True, stop=True)
            tb = sb.tile([128, HC], BF16, tag="t_nr")
            nc.vector.tensor_sub(tb[:], pb[:], b1_t[:])
            nc.sync.dma_start(xr1_loc[i * 128:(i + 1) * 128, :], tb[:])

        nc.gpsimd.collective_compute(
            "AllGather", mybir.AluOpType.bypass,
            replica_groups=[list(range(NCORES))],
            ins=[xl1_sl[:, :]], outs=[xl1_full[:, :]])

        MAXCH = 8  # dma_gather is only safe up to 1024 indices per call

        def gathers(out3, in_ap, idx_t, col0, nch, elem):
            for b0 in range(0, nch, MAXCH):
                b1 = min(b0 + MAXCH, nch)
                n = (b1 - b0) * 128
                nc.gpsimd.dma_gather(
                    out3[:, b0:b1, :], in_ap,
                    idx_t[:, col0 + b0 * 8:col0 + b1 * 8],
                    num_idxs=n, num_idxs_reg=n, elem_size=elem)

        # ---- stage 2: layer-1 edge stage per dst tile
        psx.close()
        ps = psx.enter_context(tc.tile_pool(name="ps2", bufs=2, space="PSUM"))
        for t in range(NT):
            gxl = sb.tile([128, CH, HC], BF16, tag="gxl")
            gathers(gxl[:, 0:CHA, :], xl1_full[0:BANKA, :], iA_t,
                    t * CHA * 8, CHA, HC)
            gathers(gxl[:, CHA:CH, :], xl1_full[BANKA:NPADG, :], iB_t,
                    t * CHB * 8, CHB, HC)
            gxr = sb.tile([128, CH, HC], BF16, tag="gxr")
            gathers(gxr[:, :, :], xr1_loc[:, :], iR_t, t * CH * 8, CH, HC)

            ex_t = sb.tile([128, CH, H], BF16, tag="ex")
            ind_t = sb.tile([128, CH, 128], BF16, tag="ind")
            den_p = ps.tile([128, H], F32, tag="den")
            for j in range(CH):
                z = sb.tile([128, HC], BF16, tag="z")
                nc.vector.tensor_add(z[:], gxl[:, j, :], gxr[:, j, :])
                r = sb.tile([128, HC], BF16, tag="r")
                nc.scalar.activation(r[:], z[:], AF.Relu, scale=0.8)
                s = sb.tile([128, HC], BF16, tag="s")
                nc.scalar.mul(s[:], gxl[:, j, :], 0.2)
                nc.vector.tensor_add(z[:], r[:], s[:])
                nc.vector.tensor_mul(z[:], z[:], att_t[:])
                sc = sb.tile([128, H], F32, tag="sc")
                nc.vector.tensor_reduce(
                    sc[:], z[:].rearrange("p (h c) -> p h c", h=H),
                    axis=mybir.AxisListType.X, op=OP.add)
                nc.scalar.activation(ex_t[:, j, :], sc[:], AF.Exp)
                nc.vector.tensor_scalar(
                    ind_t[:, j, :], iota_f[:], dl_t[:, t * CH + j:t * CH + j + 1],
                    None, OP.is_equal)
                nc.tensor.matmul(den_p[:], ind_t[:, j, :], ex_t[:, j, :],
                                 start=(j == 0), stop=(j == CH - 1))
            rden = sb.tile([128, H], F32, tag="rden")
            nc.vector.tensor_scalar(rden[:], den_p[:], 1e-20, None, OP.max)
            nc.vector.reciprocal(rden[:], rden[:])

            agg_p = ps.tile([128, HC], F32, tag="agg")
            for j in range(CH):
                msg = sb.tile([128, HC], BF16, tag="msg")
                nc.vector.tensor_tensor(
                    msg[:].rearrange("p (h c) -> p h c", h=H),
                    gxl[:, j, :].rearrange("p (h c) -> p h c", h=H),
                    ex_t[:, j, :].unsqueeze(2).broadcast_to([128, H, C]),
                    OP.mult)
                nc.tensor.matmul(agg_p[:], ind_t[:, j, :], msg[:],
                                 start=(j == 0), stop=(j == CH - 1))
            h1_t = sb.tile([128, HC], BF16, tag="h1")
            for h in range(H):
                nc.scalar.activation(h1_t[:, h * C:(h + 1) * C],
                                     agg_p[:, h * C:(h + 1) * C],
                                     AF.Relu, scale=rden[:, h:h + 1])
            for i in range(2):
                nc.sync.dma_start(h1c_sl[i][t * 128:(t + 1) * 128, :],
                                  h1_t[:, i * 128:(i + 1) * 128])

        # ---- stage 3: layer-2 node transforms
        psx.close()
        ps = psx.enter_context(tc.tile_pool(name="ps3", bufs=2, space="PSUM"))
        h1T = cp.tile([128, 2, RPAD], BF16)
        for i in range(2):
            nc.sync.dma_start_transpose(h1T[:, i, :], h1c_sl[i][:, :])
        for i in range(NT):
            pa = ps.tile([128, C], F32, tag="p2_nl")
            for cc in range(2):
                nc.tensor.matmul(pa[:], h1T[:, cc, i * 128:(i + 1) * 128],
                                 wl2_t[:, cc, :], start=(cc == 0), stop=(cc == 1))
            ta = sb.tile([128, C], F32, tag="t2_nl")
            nc.vector.tensor_add(ta[:], pa[:], b2_t[:])
            nc.sync.dma_start(xl2_sl[i * 128:(i + 1) * 128, :], ta[:])
            pb = ps.tile([128, C], F32, tag="p2_nr")
            for cc in range(2):
                nc.tensor.matmul(pb[:], h1T[:, cc, i * 128:(i + 1) * 128],
                                 wr2_t[:, cc, :], start=(cc == 0), stop=(cc == 1))
            tb = sb.tile([128, C], F32, tag="t2_nr")
            nc.vector.tensor_sub(tb[:], pb[:], b2_t[:])
            nc.sync.dma_start(xr2_loc[i * 128:(i + 1) * 128, :], tb[:])

        nc.gpsimd.collective_compute(
            "AllGather", mybir.AluOpType.bypass,
            replica_groups=[list(range(NCORES))],
            ins=[xl2_sl[:, :]], outs=[xl2_full[:, :]])

        # ---- stage 4: layer-2 edge stage + pooling
        psx.close()
        ps = psx.enter_context(tc.tile_pool(name="ps4", bufs=2, space="PSUM"))
        pool_acc = cp.tile([G, C], F32)
        nc.vector.memset(pool_acc[:], 0.0)
        for t in range(NT):
            gxl2 = sb.tile([128, CH, C], F32, tag="gxl2")
            gathers(gxl2[:, 0:CHA, :], xl2_full[0:BANKA, :], iA_t,
                    t * CHA * 8, CHA, C)
            gathers(gxl2[:, CHA:CH, :], xl2_full[BANKA:NPADG, :], iB_t,
                    t * CHB * 8, CHB, C)
            gxr2 = sb.tile([128, CH, C], F32, tag="gxr2")
            gathers(gxr2[:, :, :], xr2_loc[:, :], iR_t, t * CH * 8, CH, C)

            ex2_t = sb.tile([128, CH, 1], BF16, tag="ex2")
            ind2_t = sb.tile([128, CH, 128], BF16, tag="ind2")
            den2_p = ps.tile([128, 1], F32, tag="den2")
            for j in range(CH):
                z = sb.tile([128, C], BF16, tag="z2")
                nc.vector.tensor_add(z[:], gxl2[:, j, :], gxr2[:, j, :])
                r = sb.tile([128, C], BF16, tag="r2")
                nc.scalar.activation(r[:], z[:], AF.Relu, scale=0.8)
                s = sb.tile([128, C], BF16, tag="s2")
                nc.scalar.mul(s[:], gxl2[:, j, :], 0.2)
                nc.vector.tensor_add(z[:], r[:], s[:])
                nc.vector.tensor_mul(z[:], z[:], att2_t[:])
                sc = sb.tile([128, 1], F32, tag="sc2")
                nc.vector.tensor_reduce(
                    sc[:], z[:].unsqueeze(1),
                    axis=mybir.AxisListType.X, op=OP.add)
                nc.scalar.activation(ex2_t[:, j, :], sc[:], AF.Exp)
                nc.vector.tensor_scalar(
                    ind2_t[:, j, :], iota_f[:], dl_t[:, t * CH + j:t * CH + j + 1],
                    None, OP.is_equal)
                nc.tensor.matmul(den2_p[:], ind2_t[:, j, :], ex2_t[:, j, :],
                                 start=(j == 0), stop=(j == CH - 1))
            rden2 = sb.tile([128, 1], F32, tag="rden2")
            nc.vector.tensor_scalar(rden2[:], den2_p[:], 1e-20, None, OP.max)
            nc.vector.reciprocal(rden2[:], rden2[:])

            agg2_p = ps.tile([128, C], F32, tag="agg2")
            for j in range(CH):
                msg = sb.tile([128, C], BF16, tag="msg2")
                nc.vector.tensor_tensor(
                    msg[:], gxl2[:, j, :],
                    ex2_t[:, j, :].broadcast_to([128, C]), OP.mult)
                nc.tensor.matmul(agg2_p[:], ind2_t[:, j, :], msg[:],
                                 start=(j == 0), stop=(j == CH - 1))
            h2_t = sb.tile([128, C], F32, tag="h2")
            nc.scalar.mul(h2_t[:], agg2_p[:], rden2[:, 0:1])

            pool_p = ps.tile([G, C], F32, tag="poolp")
            nc.tensor.matmul(pool_p[:], poolw_t[:, t * G:(t + 1) * G], h2_t[:],
                             start=True, stop=True)
            nc.vector.tensor_add(pool_acc[:], pool_acc[:], pool_p[:])

        ot = cp.tile([G, C], F32)
        nc.vector.tensor_copy(ot[:], pool_acc[:])
        nc.sync.dma_start(out_pool[:, :], ot[:])

    nc.finalize()
    return nc


# -------------------------------------------------------------------- driver

def kernel(x, edge_index, batch, Wl1, Wr1, att1, b1, Wl2, Wr2, att2, b2,
           Wo, bo):
    from concourse.bass_utils import run_bass_kernel_spmd

    x = np.asarray(x, np.float32)
    edge_index = np.asarray(edge_index)
    batch = np.asarray(batch)
    Wl1 = np.asarray(Wl1, np.float32); Wr1 = np.asarray(Wr1, np.float32)
    att1 = np.asarray(att1, np.float32); b1 = np.asarray(b1, np.float32)
    Wl2 = np.asarray(Wl2, np.float32); Wr2 = np.asarray(Wr2, np.float32)
    att2 = np.asarray(att2, np.float32); b2 = np.asarray(b2, np.float32)
    Wo = np.asarray(Wo, np.float32); bo = np.asarray(bo, np.float32)

    cores, poolw, CHA, CHB = _preprocess(x, edge_index, batch)

    key = (CHA, CHB)
    if key not in _CACHE:
        _CACHE[key] = _build_nc(CHA, CHB)
    nc = _CACHE[key]

    b1rep = np.tile(b1.reshape(1, HC), (128, 1)).astype(np.float32)
    b2rep = np.tile(b2.reshape(1, C), (128, 1)).astype(np.float32)
    attrep = np.tile(att1.reshape(1, HC), (128, 1)).astype(BF)
    att2rep = np.tile(att2.reshape(1, C), (128, 1)).astype(BF)
    wl2 = Wl2.reshape(2, 128, C).transpose(1, 0, 2).astype(BF)
    wr2 = Wr2.reshape(2, 128, C).transpose(1, 0, 2).astype(BF)
    wl1 = Wl1.astype(BF); wr1 = Wr1.astype(BF)

    xpad = np.zeros((NCORES, RPAD, F_IN), BF)
    xr = x.reshape(NCORES, RP, F_IN)
    xpad[:, :RP, :] = xr.astype(BF)

    in_maps = []
    for c in range(NCORES):
        in_maps.append(dict(
            x_sl=xpad[c], wl1=wl1, wr1=wr1, wl2=wl2, wr2=wr2,
            b1rep=b1rep, b2rep=b2rep, attrep=attrep, att2rep=att2rep,
            idxA=cores[c]["idxA"], idxB=cores[c]["idxB"],
            idxR=cores[c]["idxR"], dl=cores[c]["dl"].astype(np.float32),
            poolw=poolw[c].astype(np.float32),
        ))
    res = run_bass_kernel_spmd(nc, in_maps, core_ids=list(range(NCORES)))
    pooled = np.zeros((G, C), np.float32)
    for c in range(NCORES):
        pooled += np.asarray(res.results[c]["out_pool"])
    return (pooled @ Wo + bo).astype(np.float32)


# revision 3
# speedup vs baseline: 14.9354x; 14.9354x over previous
"""GATv2 (2 layers) + mean-pool + linear head on 8 Trainium2 NeuronCores.

Sharding: destination nodes are range-partitioned across the 8 cores
(6250 nodes each, padded to 6272 = 49*128). Edges (with self-loops) are
sorted by destination and assigned to the owner of their dst. Per core:

  1. node transforms xl=x@Wl+b, xr=x@Wr-b for the local node slice (PE),
  2. AllGather of xl (source-side transform) so any core can gather any
     source row,
  3. per 128-dst tile: dma_gather of per-edge source rows (split in two
     index banks because gather indices are int16), per-edge scores via
     DVE/ACT, per-dst softmax denominator + weighted message aggregation
     via indicator matmuls on the PE (edges of a tile only reference the
     tile's 128 dsts), normalization folded into the psum read-out.

Softmax is computed without the segment-max shift (scores are O(1); the
shift cancels exactly) and per-dst score terms are dropped (they cancel
in the softmax too). leaky_relu(z) = relu(0.8 z) + 0.2 z with the 0.2*xr
part dropped (per-dst) and 0.2*xl kept.

The mean-pool + final linear run as a tiny per-core [8,64] partial
(indicator matmul with 1/count weights) summed on host.
"""

import sys
import numpy as np

for _p in ("/opt/trn_rl_repo", "/root/.axon_site/_ro/trn_rl_repo"):
    if _p not in sys.path:
        sys.path.insert(0, _p)

import ml_dtypes

BF = ml_dtypes.bfloat16

# Problem constants
N, E, F_IN, H, C, G = 50000, 800000, 128, 4, 64, 8
HC = H * C                      # 256
NCORES = 8
RP = N // NCORES                # 6250 rows per core
RPAD = 6272                     # 49*128
NT = RPAD // 128                # 49 dst/node tiles per core
NPADG = NCORES * RPAD           # 50176 padded global rows
BANKA = 5 * RPAD                # 31360; int16 gather bank split

_CACHE = {}


# ----------------------------------------------------------------- host prep

def _wrap16_rows(a):
    """[T, n] int16 -> [16, T*n//16] in dma_gather index layout per row."""
    T, n = a.shape
    return a.reshape(T, n // 16, 16).transpose(2, 0, 1).reshape(16, T * n // 16)


def _prep_core(sp_all, dl_all, tile_of, CHA, CHB):
    """Build padded per-tile index/dst arrays for one core.

    sp_all: global padded src row per edge (sorted by dst)
    dl_all: dst local row (0..6271) per edge
    tile_of: dl_all // 128
    """
    CH = CHA + CHB
    nA, nB = CHA * 128, CHB * 128
    bank_b = sp_all >= BANKA
    # stable order by (tile, bank)
    key = tile_of * 2 + bank_b
    order = np.argsort(key, kind="stable")
    sp = sp_all[order]
    dl = dl_all[order] - tile_of[order] * 128
    keys = key[order]
    cnt = np.bincount(keys, minlength=2 * NT)
    # position of each edge inside its (tile, bank) group
    starts = np.concatenate([[0], np.cumsum(cnt)[:-1]])
    pos = np.arange(len(sp)) - np.repeat(starts, cnt)
    grp_t = keys >> 1
    grp_b = keys & 1
    assert cnt[0::2].max(initial=0) <= nA, "bank-A overflow; raise CHA"
    assert cnt[1::2].max(initial=0) <= nB, "bank-B overflow; raise CHB"

    idxA = np.zeros((NT, nA), np.int16)
    idxB = np.zeros((NT, nB), np.int16)
    dlp = np.full((NT, CH * 128), -1.0, np.float32)
    a = grp_b == 0
    idxA[grp_t[a], pos[a]] = sp[a].astype(np.int16)
    b = ~a
    idxB[grp_t[b], pos[b]] = (sp[b] - BANKA).astype(np.int16)
    dlp[grp_t[a], pos[a]] = dl[a]
    dlp[grp_t[b], nA + pos[b]] = dl[b]

    idxR = np.where(dlp >= 0, dlp + (np.arange(NT) * 128)[:, None], 0).astype(np.int16)
    # device layouts
    return dict(
        idxA=_wrap16_rows(idxA),
        idxB=_wrap16_rows(idxB),
        idxR=_wrap16_rows(idxR),
        # dl[p, t*CH + j] = dst_local of edge (t, j*128+p)
        dl=dlp.reshape(NT, CH, 128).transpose(2, 0, 1).reshape(128, NT * CH),
    )


def _preprocess(x, edge_index, batch):
    src = np.concatenate([edge_index[0].astype(np.int64),
                          np.arange(N, dtype=np.int64)])
    dst = np.concatenate([edge_index[1].astype(np.int64),
                          np.arange(N, dtype=np.int64)])
    order = np.argsort(dst, kind="stable")
    srcs = src[order]
    dsts = dst[order]
    sp_all = srcs + 22 * (srcs // RP)          # padded global row
    core_lo = np.searchsorted(dsts, np.arange(NCORES + 1) * RP)

    # global uniform chunk counts
    tile_gl = (dsts - (dsts // RP) * RP) // 128 + (dsts // RP) * NT
    bank_b = (sp_all >= BANKA).astype(np.int64)
    cnt = np.bincount(tile_gl * 2 + bank_b, minlength=2 * NCORES * NT)
    CHA = int(-(-cnt[0::2].max() // 128))
    CHB = int(-(-cnt[1::2].max() // 128))

    cores = []
    for c in range(NCORES):
        lo, hi = core_lo[c], core_lo[c + 1]
        dl_all = dsts[lo:hi] - c * RP
        cores.append(_prep_core(sp_all[lo:hi], dl_all, dl_all // 128, CHA, CHB))

    # mean-pool weights: [N, G] one-hot / count, padded + tile-major
    cntg = np.bincount(batch.astype(np.int64), minlength=G).astype(np.float32)
    w = np.zeros((NCORES * RPAD, G), np.float32)
    rows = np.arange(N) + 22 * (np.arange(N) // RP)
    w[rows, batch.astype(np.int64)] = 1.0 / np.maximum(cntg, 1.0)[batch.astype(np.int64)]
    poolw = w.reshape(NCORES, NT, 128, G).transpose(0, 2, 1, 3).reshape(
        NCORES, 128, NT * G)
    return cores, poolw, CHA, CHB


# ---------------------------------------------------------------- bass build

def _build_nc(CHA, CHB):
    from contextlib import ExitStack
    from concourse import bacc, mybir
    from concourse import tile

    F32 = mybir.dt.float32
    BF16 = mybir.dt.bfloat16
    I16 = mybir.dt.int16
    AF = mybir.ActivationFunctionType
    OP = mybir.AluOpType
    CH = CHA + CHB

    nc = bacc.Bacc(None, target_bir_lowering=False, debug=False)
    dp = nc.declare_dram_parameter
    x_sl = dp("x_sl", [RPAD, F_IN], BF16, isOutput=False)
    wl1 = dp("wl1", [F_IN, HC], BF16, isOutput=False)
    wr1 = dp("wr1", [F_IN, HC], BF16, isOutput=False)
    wl2 = dp("wl2", [128, 2, C], BF16, isOutput=False)
    wr2 = dp("wr2", [128, 2, C], BF16, isOutput=False)
    b1rep = dp("b1rep", [128, HC], F32, isOutput=False)
    b2rep = dp("b2rep", [128, C], F32, isOutput=False)
    attrep = dp("attrep", [128, HC], BF16, isOutput=False)
    att2rep = dp("att2rep", [128, C], BF16, isOutput=False)
    idxA_d = dp("idxA", [16, NT * CHA * 8], I16, isOutput=False)
    idxB_d = dp("idxB", [16, NT * CHB * 8], I16, isOutput=False)
    idxR_d = dp("idxR", [16, NT * CH * 8], I16, isOutput=False)
    dl_d = dp("dl", [128, NT * CH], F32, isOutput=False)
    poolw_d = dp("poolw", [128, NT * G], F32, isOutput=False)
    out_pool = dp("out_pool", [G, C], F32, isOutput=True)

    xl1_sl = nc.dram_tensor("xl1_sl", [RPAD, HC], BF16)
    xr1_loc = nc.dram_tensor("xr1_loc", [RPAD, HC], BF16)
    xl1_full = nc.dram_tensor("xl1_full", [NPADG, HC], BF16, addr_space="Shared")
    h1c_sl = [nc.dram_tensor(f"h1c{i}_sl", [RPAD, 128], BF16) for i in range(2)]
    xl2_sl = nc.dram_tensor("xl2_sl", [RPAD, C], F32)
    xr2_loc = nc.dram_tensor("xr2_loc", [RPAD, C], F32)
    xl2_full = nc.dram_tensor("xl2_full", [NPADG, C], F32, addr_space="Shared")

    with tile.TileContext(nc) as tc, ExitStack() as ctx:
        cp = ctx.enter_context(tc.tile_pool(name="consts", bufs=1))
        sb = ctx.enter_context(tc.tile_pool(name="work", bufs=2))

        def cload(name, src_ap, shape, dtype):
            t = cp.tile(shape, dtype, tag=name)
            nc.sync.dma_start(t[:], src_ap)
            return t

        wl1_t = cload("wl1c", wl1[:, :], [F_IN, HC], BF16)
        wr1_t = cload("wr1c", wr1[:, :], [F_IN, HC], BF16)
        wl2_t = cload("wl2c", wl2[:, :, :], [128, 2, C], BF16)
        wr2_t = cload("wr2c", wr2[:, :, :], [128, 2, C], BF16)
        b1_t = cload("b1c", b1rep[:, :], [128, HC], F32)
        b2_t = cload("b2c", b2rep[:, :], [128, C], F32)
        att_t = cload("attc", attrep[:, :], [128, HC], BF16)
        att2_t = cload("att2c", att2rep[:, :], [128, C], BF16)
        dl_t = cload("dlc", dl_d[:, :], [128, NT * CH], F32)
        poolw_t = cload("poolwc", poolw_d[:, :], [128, NT * G], F32)

        iota_i = cp.tile([128, 128], mybir.dt.int32)
        nc.gpsimd.iota(iota_i[:], pattern=[[1, 128]], base=0, channel_multiplier=0)
        iota_f = cp.tile([128, 128], F32)
        nc.vector.tensor_copy(iota_f[:], iota_i[:])

        iA_t = cp.tile([128, NT * CHA * 8], I16)
        iB_t = cp.tile([128, NT * CHB * 8], I16)
        iR_t = cp.tile([128, NT * CH * 8], I16)
        for k in range(8):
            nc.sync.dma_start(iA_t[16 * k:16 * (k + 1), :], idxA_d[:, :])
            nc.sync.dma_start(iB_t[16 * k:16 * (k + 1), :], idxB_d[:, :])
            nc.sync.dma_start(iR_t[16 * k:16 * (k + 1), :], idxR_d[:, :])

        # ---- stage 1: x^T and layer-1 node transforms
        xT = cp.tile([128, RPAD], BF16)
        nc.sync.dma_start_transpose(xT[:], x_sl[:, :])
        psx = ExitStack()
        ctx.callback(psx.close)
        ps = psx.enter_context(tc.tile_pool(name="ps1", bufs=2, space="PSUM"))
        for i in range(NT):
            pa = ps.tile([128, HC], F32, tag="p_nl")
            nc.tensor.matmul(pa[:], xT[:, i * 128:(i + 1) * 128], wl1_t[:],
                             start=True, stop=True)
            ta = sb.tile([128, HC], BF16, tag="t_nl")
            nc.vector.tensor_add(ta[:], pa[:], b1_t[:])
            nc.sync.dma_start(xl1_sl[i * 128:(i + 1) * 128, :], ta[:])
            pb = ps.tile([128, HC], F32, tag="p_nr")
            nc.tensor.matmul(pb[:], xT[:, i * 128:(i + 1) * 128], wr1_t[:],
                             start=True, stop=True)
            tb = sb.tile([128, HC], BF16, tag="t_nr")
            nc.vector.tensor_sub(tb[:], pb[:], b1_t[:])
            nc.sync.dma_start(xr1_loc[i * 128:(i + 1) * 128, :], tb[:])

        nc.gpsimd.collective_compute(
            "AllGather", mybir.AluOpType.bypass,
            replica_groups=[list(range(NCORES))],
            ins=[xl1_sl[:, :]], outs=[xl1_full[:, :]])

        MAXCH = 8  # dma_gather is only safe up to 1024 indices per call

        def gathers(out3, in_ap, idx_t, col0, nch, elem):
            for b0 in range(0, nch, MAXCH):
                b1 = min(b0 + MAXCH, nch)
                n = (b1 - b0) * 128
                nc.gpsimd.dma_gather(
                    out3[:, b0:b1, :], in_ap,
                    idx_t[:, col0 + b0 * 8:col0 + b1 * 8],
                    num_idxs=n, num_idxs_reg=n, elem_size=elem)

        # ---- stage 2: layer-1 edge stage per dst tile
        psx.close()
        ps = psx.enter_context(tc.tile_pool(name="ps2", bufs=2, space="PSUM"))
        for t in range(NT):
            gxl = sb.tile([128, CH, HC], BF16, tag="gxl")
            gathers(gxl[:, 0:CHA, :], xl1_full[0:BANKA, :], iA_t,
                    t * CHA * 8, CHA, HC)
            gathers(gxl[:, CHA:CH, :], xl1_full[BANKA:NPADG, :], iB_t,
                    t * CHB * 8, CHB, HC)
            gxr = sb.tile([128, CH, HC], BF16, tag="gxr")
            gathers(gxr[:, :, :], xr1_loc[:, :], iR_t, t * CH * 8, CH, HC)

            ex_t = sb.tile([128, CH, H], BF16, tag="ex")
            ind_t = sb.tile([128, CH, 128], BF16, tag="ind")
            den_p = ps.tile([128, H], F32, tag="den")
            z = sb.tile([128, CH, HC], BF16, tag="z", bufs=1)
            nc.vector.tensor_add(z[:], gxl[:], gxr[:])
            r = sb.tile([128, CH, HC], BF16, tag="r", bufs=1)
            nc.scalar.activation(r[:], z[:], AF.Relu, scale=0.8)
            nc.scalar.mul(z[:], gxl[:], 0.2)
            nc.vector.tensor_add(r[:], r[:], z[:])
            nc.vector.tensor_tensor(
                z[:], r[:],
                att_t[:].unsqueeze(1).broadcast_to([128, CH, HC]), OP.mult)
            sc = sb.tile([128, CH, H], F32, tag="sc")
            nc.vector.tensor_reduce(
                sc[:], z[:].rearrange("p t (h c) -> p t h c", h=H),
                axis=mybir.AxisListType.X, op=OP.add)
            nc.scalar.activation(ex_t[:], sc[:], AF.Exp)
            nc.vector.tensor_tensor(
                ind_t[:], iota_f[:].unsqueeze(1).broadcast_to([128, CH, 128]),
                dl_t[:, t * CH:(t + 1) * CH].unsqueeze(2).broadcast_to(
                    [128, CH, 128]), OP.is_equal)
            for j in range(CH):
                nc.tensor.matmul(den_p[:], ind_t[:, j, :], ex_t[:, j, :],
                                 start=(j == 0), stop=(j == CH - 1))
            rden = sb.tile([128, H], F32, tag="rden")
            nc.vector.tensor_scalar(rden[:], den_p[:], 1e-20, None, OP.max)
            nc.vector.reciprocal(rden[:], rden[:])

            agg_p = ps.tile([128, HC], F32, tag="agg")
            msg = sb.tile([128, CH, HC], BF16, tag="msg", bufs=1)
            nc.vector.tensor_tensor(
                msg[:].rearrange("p t (h c) -> p t h c", h=H),
                gxl[:].rearrange("p t (h c) -> p t h c", h=H),
                ex_t[:].unsqueeze(3).broadcast_to([128, CH, H, C]), OP.mult)
            for j in range(CH):
                nc.tensor.matmul(agg_p[:], ind_t[:, j, :], msg[:, j, :],
                                 start=(j == 0), stop=(j == CH - 1))
            h1_t = sb.tile([128, HC], BF16, tag="h1")
            for h in range(H):
                nc.scalar.activation(h1_t[:, h * C:(h + 1) * C],
                                     agg_p[:, h * C:(h + 1) * C],
                                     AF.Relu, scale=rden[:, h:h + 1])
            for i in range(2):
                nc.sync.dma_start(h1c_sl[i][t * 128:(t + 1) * 128, :],
                                  h1_t[:, i * 128:(i + 1) * 128])

        # ---- stage 3: layer-2 node transforms
        psx.close()
        ps = psx.enter_context(tc.tile_pool(name="ps3", bufs=2, space="PSUM"))
        h1T = cp.tile([128, 2, RPAD], BF16)
        for i in range(2):
            nc.sync.dma_start_transpose(h1T[:, i, :], h1c_sl[i][:, :])
        for i in range(NT):
            pa = ps.tile([128, C], F32, tag="p2_nl")
            for cc in range(2):
                nc.tensor.matmul(pa[:], h1T[:, cc, i * 128:(i + 1) * 128],
                                 wl2_t[:, cc, :], start=(cc == 0), stop=(cc == 1))
            ta = sb.tile([128, C], F32, tag="t2_nl")
            nc.vector.tensor_add(ta[:], pa[:], b2_t[:])
            nc.sync.dma_start(xl2_sl[i * 128:(i + 1) * 128, :], ta[:])
            pb = ps.tile([128, C], F32, tag="p2_nr")
            for cc in range(2):
                nc.tensor.matmul(pb[:], h1T[:, cc, i * 128:(i + 1) * 128],
                                 wr2_t[:, cc, :], start=(cc == 0), stop=(cc == 1))
            tb = sb.tile([128, C], F32, tag="t2_nr")
            nc.vector.tensor_sub(tb[:], pb[:], b2_t[:])
            nc.sync.dma_start(xr2_loc[i * 128:(i + 1) * 128, :], tb[:])

        nc.gpsimd.collective_compute(
            "AllGather", mybir.AluOpType.bypass,
            replica_groups=[list(range(NCORES))],
            ins=[xl2_sl[:, :]], outs=[xl2_full[:, :]])

        # ---- stage 4: layer-2 edge stage + pooling
        psx.close()
        ps = psx.enter_context(tc.tile_pool(name="ps4", bufs=2, space="PSUM"))
        pool_acc = cp.tile([G, C], F32)
        nc.vector.memset(pool_acc[:], 0.0)
        for t in range(NT):
            gxl2 = sb.tile([128, CH, C], F32, tag="gxl2")
            gathers(gxl2[:, 0:CHA, :], xl2_full[0:BANKA, :], iA_t,
                    t * CHA * 8, CHA, C)
            gathers(gxl2[:, CHA:CH, :], xl2_full[BANKA:NPADG, :], iB_t,
                    t * CHB * 8, CHB, C)
            gxr2 = sb.tile([128, CH, C], F32, tag="gxr2")
            gathers(gxr2[:, :, :], xr2_loc[:, :], iR_t, t * CH * 8, CH, C)

            ex2_t = sb.tile([128, CH, 1], BF16, tag="ex2")
            ind2_t = sb.tile([128, CH, 128], BF16, tag="ind2")
            den2_p = ps.tile([128, 1], F32, tag="den2")
            z = sb.tile([128, CH, C], BF16, tag="z2", bufs=1)
            nc.vector.tensor_add(z[:], gxl2[:], gxr2[:])
            r = sb.tile([128, CH, C], BF16, tag="r2", bufs=1)
            nc.scalar.activation(r[:], z[:], AF.Relu, scale=0.8)
            nc.scalar.mul(z[:], gxl2[:], 0.2)
            nc.vector.tensor_add(r[:], r[:], z[:])
            nc.vector.tensor_tensor(
                z[:], r[:],
                att2_t[:].unsqueeze(1).broadcast_to([128, CH, C]), OP.mult)
            sc = sb.tile([128, CH, 1], F32, tag="sc2")
            nc.vector.tensor_reduce(
                sc[:], z[:].unsqueeze(2),
                axis=mybir.AxisListType.X, op=OP.add)
            nc.scalar.activation(ex2_t[:], sc[:], AF.Exp)
            nc.vector.tensor_tensor(
                ind2_t[:], iota_f[:].unsqueeze(1).broadcast_to([128, CH, 128]),
                dl_t[:, t * CH:(t + 1) * CH].unsqueeze(2).broadcast_to(
                    [128, CH, 128]), OP.is_equal)
            for j in range(CH):
                nc.tensor.matmul(den2_p[:], ind2_t[:, j, :], ex2_t[:, j, :],
                                 start=(j == 0), stop=(j == CH - 1))
            rden2 = sb.tile([128, 1], F32, tag="rden2")
            nc.vector.tensor_scalar(rden2[:], den2_p[:], 1e-20, None, OP.max)
            nc.vector.reciprocal(rden2[:], rden2[:])

            agg2_p = ps.tile([128, C], F32, tag="agg2")
            msg = sb.tile([128, CH, C], BF16, tag="msg2", bufs=1)
            nc.vector.tensor_tensor(
                msg[:], gxl2[:],
                ex2_t[:].broadcast_to([128, CH, C]), OP.mult)
            for j in range(CH):
                nc.tensor.matmul(agg2_p[:], ind2_t[:, j, :], msg[:, j, :],
                                 start=(j == 0), stop=(j == CH - 1))
            h2_t = sb.tile([128, C], F32, tag="h2")
            nc.scalar.mul(h2_t[:], agg2_p[:], rden2[:, 0:1])

            pool_p = ps.tile([G, C], F32, tag="poolp")
            nc.tensor.matmul(pool_p[:], poolw_t[:, t * G:(t + 1) * G], h2_t[:],
                             start=True, stop=True)
            nc.vector.tensor_add(pool_acc[:], pool_acc[:], pool_p[:])

        ot = cp.tile([G, C], F32)
        nc.vector.tensor_copy(ot[:], pool_acc[:])
        nc.sync.dma_start(out_pool[:, :], ot[:])

    nc.finalize()
    return nc


# -------------------------------------------------------------------- driver

def kernel(x, edge_index, batch, Wl1, Wr1, att1, b1, Wl2, Wr2, att2, b2,
           Wo, bo):
    from concourse.bass_utils import run_bass_kernel_spmd

    x = np.asarray(x, np.float32)
    edge_index = np.asarray(edge_index)
    batch = np.asarray(batch)
    Wl1 = np.asarray(Wl1, np.float32); Wr1 = np.asarray(Wr1, np.float32)
    att1 = np.asarray(att1, np.float32); b1 = np.asarray(b1, np.float32)
    Wl2 = np.asarray(Wl2, np.float32); Wr2 = np.asarray(Wr2, np.float32)
    att2 = np.asarray(att2, np.float32); b2 = np.asarray(b2, np.float32)
    Wo = np.asarray(Wo, np.float32); bo = np.asarray(bo, np.float32)

    cores, poolw, CHA, CHB = _preprocess(x, edge_index, batch)

    key = (CHA, CHB)
    if key not in _CACHE:
        _CACHE[key] = _build_nc(CHA, CHB)
    nc = _CACHE[key]

    b1rep = np.tile(b1.reshape(1, HC), (128, 1)).astype(np.float32)
    b2rep = np.tile(b2.reshape(1, C), (128, 1)).astype(np.float32)
    attrep = np.tile(att1.reshape(1, HC), (128, 1)).astype(BF)
    att2rep = np.tile(att2.reshape(1, C), (128, 1)).astype(BF)
    wl2 = Wl2.reshape(2, 128, C).transpose(1, 0, 2).astype(BF)
    wr2 = Wr2.reshape(2, 128, C).transpose(1, 0, 2).astype(BF)
    wl1 = Wl1.astype(BF); wr1 = Wr1.astype(BF)

    xpad = np.zeros((NCORES, RPAD, F_IN), BF)
    xr = x.reshape(NCORES, RP, F_IN)
    xpad[:, :RP, :] = xr.astype(BF)

    in_maps = []
    for c in range(NCORES):
        in_maps.append(dict(
            x_sl=xpad[c], wl1=wl1, wr1=wr1, wl2=wl2, wr2=wr2,
            b1rep=b1rep, b2rep=b2rep, attrep=attrep, att2rep=att2rep,
            idxA=cores[c]["idxA"], idxB=cores[c]["idxB"],
            idxR=cores[c]["idxR"], dl=cores[c]["dl"].astype(np.float32),
            poolw=poolw[c].astype(np.float32),
        ))
    res = run_bass_kernel_spmd(nc, in_maps, core_ids=list(range(NCORES)))
    pooled = np.zeros((G, C), np.float32)
    for c in range(NCORES):
        pooled += np.asarray(res.results[c]["out_pool"])
    return (pooled @ Wo + bo).astype(np.float32)
